# revision 3
# baseline (speedup 1.0000x reference)
"""Single-head attention (B=4, N=2048, D=1024, fp32 I/O) on 8 TRN2 NeuronCores.

Sharding: data-parallel over (batch, sequence-half): core i handles batch i//2,
query rows (i%2)*1024:(i%2+1)*1024.  No collectives — each core receives the
full 2048 keys of its batch (its own query rows permuted first; attention is
permutation-invariant over keys) and computes k/v projections locally.

On-device per core:
  xT  = transpose(x) in bf16      (DMA-cast f32->bf16, then xbar DMA-transpose)
  WqT/WkT/WvT similarly
  qT[d,n] = Wq @ x^T + bq         (TensorE, bf16 in / f32 psum, bias via ACT)
  kT[d,m], v[m,d] likewise        (v bias added on DVE eviction)
  per 128-row query block: S = q@k^T (psum), P = exp(S/32) (ACT, accum sums),
  P^T via xbar DMA-transpose, out = (P^T)^T @ v * (1/rowsum) (TensorE + DVE).
"""

import numpy as np

import concourse.bass as bass
import concourse.bacc as bacc
import concourse.mybir as mybir
import concourse.tile as tile
from concourse.bass_utils import run_bass_kernel_spmd

B, N, D = 4, 2048, 1024
P = 128
NCORES = 8
HALF = N // 2              # 1024 query rows per core
SCALE = float(D) ** -0.5   # 1/32

F32 = mybir.dt.float32
BF16 = mybir.dt.bfloat16


def build_nc():
    nc = bacc.Bacc("TRN2", target_bir_lowering=False)

    x_h = nc.declare_dram_parameter("x", [N, D], F32, isOutput=False)
    wq_h = nc.declare_dram_parameter("wq", [D, D], F32, isOutput=False)
    wk_h = nc.declare_dram_parameter("wk", [D, D], F32, isOutput=False)
    wv_h = nc.declare_dram_parameter("wv", [D, D], F32, isOutput=False)
    bqt_h = nc.declare_dram_parameter("bqt", [P, 8], F32, isOutput=False)
    bkt_h = nc.declare_dram_parameter("bkt", [P, 8], F32, isOutput=False)
    bv_h = nc.declare_dram_parameter("bv", [1, D], F32, isOutput=False)
    out_h = nc.declare_dram_parameter("out", [HALF, D], F32, isOutput=True)

    Exp = mybir.ActivationFunctionType.Exp
    Ident = mybir.ActivationFunctionType.Identity
    AX = mybir.AxisListType.X
    ADD = mybir.AluOpType.add

    with (
        tile.TileContext(nc) as tc,
        tc.tile_pool(name="singles", bufs=1) as singles,
        tc.tile_pool(name="stage", bufs=3) as stage,
        tc.tile_pool(name="pwork", bufs=2) as pwork,
        tc.tile_pool(name="psB", bufs=2, space="PSUM") as psB,
        tc.tile_pool(name="psS", bufs=1, space="PSUM") as psS,
        tc.tile_pool(name="psO", bufs=2, space="PSUM") as psO,
    ):
        # ---- persistent SBUF tensors ----
        # xT[p, rb, j, nn] = x[rb*128+nn, j*128+p]   (x^T, c-major tiles)
        xT = singles.tile([P, 16, 8, P], BF16)
        # wT[p, dc, j, dd] = W[dc*128+dd, j*128+p]   (W^T)
        wqT = singles.tile([P, 8, 8, P], BF16)
        wkT = singles.tile([P, 8, 8, P], BF16)
        wvT = singles.tile([P, 8, 8, P], BF16)
        # qT[p, dc, n] = q[n, dc*128+p];  kT same over all 2048 keys
        qT = singles.tile([P, 8, HALF], BF16)
        kT = singles.tile([P, 8, N], BF16)
        # v[p, mc, d] = v[mc*128+p, d]
        vv = singles.tile([P, 16, D], BF16)
        vb = singles.tile([P, D], BF16)      # bv broadcast to all partitions
        bqt = singles.tile([P, 8], F32)
        bkt = singles.tile([P, 8], F32)

        # ---- stage A: load biases, cast+transpose x and W ----
        nc.sync.dma_start(out=bqt[:], in_=bqt_h[:, :])
        nc.sync.dma_start(out=bkt[:], in_=bkt_h[:, :])
        bv_ap = bv_h[:, :]
        bv_bcast = bass.AP(
            tensor=bv_ap.tensor,
            offset=bv_ap.offset,
            ap=[[0, P]] + list(bv_ap.ap[1:]),
        )
        nc.gpsimd.dma_start(out=vb[:], in_=bv_bcast)  # f32 -> bf16 cast

        for rb in range(16):
            xbf = stage.tile([P, D], BF16, tag="stg")
            nc.gpsimd.dma_start(out=xbf[:], in_=x_h[rb * P : (rb + 1) * P, :])
            nc.sync.dma_start_transpose(out=xT[:, rb, :, :], in_=xbf[:])

        for wh, wt in ((wq_h, wqT), (wk_h, wkT), (wv_h, wvT)):
            for rb in range(8):
                wbf = stage.tile([P, D], BF16, tag="stg")
                nc.gpsimd.dma_start(out=wbf[:], in_=wh[rb * P : (rb + 1) * P, :])
                nc.sync.dma_start_transpose(out=wt[:, rb, :, :], in_=wbf[:])

        # ---- stage B: projections ----
        # qT: out[d-block, n-512-half]
        for dc in range(8):
            for h2 in range(2):
                ps = psB.tile([P, 512], F32, tag="psb")
                for cc in range(8):
                    nc.tensor.matmul(
                        ps[:],
                        lhsT=wqT[:, dc, cc, :],
                        rhs=xT[:, h2 * 4 : (h2 + 1) * 4, cc, :],
                        start=(cc == 0),
                        stop=(cc == 7),
                    )
                nc.scalar.activation(
                    out=qT[:, dc, h2 * 512 : (h2 + 1) * 512],
                    in_=ps[:],
                    func=Ident,
                    bias=bqt[:, dc : dc + 1],
                    scale=1.0,
                )

        # kT: all 2048 keys
        for dc in range(8):
            for mq in range(4):
                ps = psB.tile([P, 512], F32, tag="psb")
                for cc in range(8):
                    nc.tensor.matmul(
                        ps[:],
                        lhsT=wkT[:, dc, cc, :],
                        rhs=xT[:, mq * 4 : (mq + 1) * 4, cc, :],
                        start=(cc == 0),
                        stop=(cc == 7),
                    )
                nc.scalar.activation(
                    out=kT[:, dc, mq * 512 : (mq + 1) * 512],
                    in_=ps[:],
                    func=Ident,
                    bias=bkt[:, dc : dc + 1],
                    scale=1.0,
                )

        # v: natural layout [m, d]
        for mc in range(16):
            for dh in range(2):
                ps = psB.tile([P, 512], F32, tag="psb")
                for cc in range(8):
                    nc.tensor.matmul(
                        ps[:],
                        lhsT=xT[:, mc, cc, :],
                        rhs=wvT[:, dh * 4 : (dh + 1) * 4, cc, :],
                        start=(cc == 0),
                        stop=(cc == 7),
                    )
                nc.vector.tensor_tensor(
                    out=vv[:, mc, dh * 512 : (dh + 1) * 512],
                    in0=ps[:],
                    in1=vb[:, dh * 512 : (dh + 1) * 512],
                    op=ADD,
                )

        # ---- stage C: attention, one 128-query block at a time ----
        for nb in range(8):
            S = psS.tile([P, N], F32, tag="S")  # 4 psum banks
            for mq in range(4):
                for dc in range(8):
                    nc.tensor.matmul(
                        S[:, mq * 512 : (mq + 1) * 512],
                        lhsT=qT[:, dc, nb * P : (nb + 1) * P],
                        rhs=kT[:, dc, mq * 512 : (mq + 1) * 512],
                        start=(dc == 0),
                        stop=(dc == 7),
                    )

            Pt = pwork.tile([P, N], BF16, tag="P")
            sums = pwork.tile([P, 4], F32, tag="sums")
            for mq in range(4):
                nc.scalar.activation(
                    out=Pt[:, mq * 512 : (mq + 1) * 512],
                    in_=S[:, mq * 512 : (mq + 1) * 512],
                    func=Exp,
                    scale=SCALE,
                    accum_out=sums[:, mq : mq + 1],
                )
            den = pwork.tile([P, 1], F32, tag="den")
            nc.vector.tensor_reduce(out=den[:], in_=sums[:], axis=AX, op=ADD)
            recip = pwork.tile([P, 1], F32, tag="recip")
            nc.vector.reciprocal(recip[:], den[:])

            PT = pwork.tile([P, 16, P], BF16, tag="PT")
            nc.sync.dma_start_transpose(out=PT[:], in_=Pt[:])

            outsb = pwork.tile([P, D], F32, tag="outsb")
            for dh in range(2):
                po = psO.tile([P, 512], F32, tag="po")
                for mc in range(16):
                    nc.tensor.matmul(
                        po[:],
                        lhsT=PT[:, mc, :],
                        rhs=vv[:, mc, dh * 512 : (dh + 1) * 512],
                        start=(mc == 0),
                        stop=(mc == 15),
                    )
                nc.vector.tensor_scalar_mul(
                    out=outsb[:, dh * 512 : (dh + 1) * 512],
                    in0=po[:],
                    scalar1=recip[:],
                )
            nc.sync.dma_start(
                out=out_h[nb * P : (nb + 1) * P, :],
                in_=outsb[:],
            )

    nc.finalize()
    return nc


def make_in_maps(x, Wq, bq, Wk, bk, Wv, bv):
    x = np.asarray(x, np.float32)
    Wq = np.ascontiguousarray(np.asarray(Wq, np.float32))
    Wk = np.ascontiguousarray(np.asarray(Wk, np.float32))
    Wv = np.ascontiguousarray(np.asarray(Wv, np.float32))
    bqt = np.ascontiguousarray(np.asarray(bq, np.float32).reshape(8, P).T)
    bkt = np.ascontiguousarray(np.asarray(bk, np.float32).reshape(8, P).T)
    bvr = np.ascontiguousarray(np.asarray(bv, np.float32).reshape(1, D))
    in_maps = []
    for i in range(NCORES):
        b, h = divmod(i, 2)
        xb = x[b]
        xp = np.ascontiguousarray(
            np.concatenate(
                [xb[h * HALF : (h + 1) * HALF], xb[(1 - h) * HALF : (2 - h) * HALF]],
                axis=0,
            )
        )
        in_maps.append(
            {
                "x": xp,
                "wq": Wq,
                "wk": Wk,
                "wv": Wv,
                "bqt": bqt,
                "bkt": bkt,
                "bv": bvr,
            }
        )
    return in_maps


def gather_out(results):
    out = np.empty((B, N, D), np.float32)
    for i in range(NCORES):
        b, h = divmod(i, 2)
        out[b, h * HALF : (h + 1) * HALF] = results[i]["out"]
    return out


def kernel(x, Wq, bq, Wk, bk, Wv, bv):
    nc = build_nc()
    in_maps = make_in_maps(x, Wq, bq, Wk, bk, Wv, bv)
    res = run_bass_kernel_spmd(nc, in_maps, core_ids=list(range(NCORES)))
    return gather_out(res.results)
